# revision 22
# baseline (speedup 1.0000x reference)
"""Single-head causal attention (B=8, T=2048, C=1024, H=64) on 8 TRN2 NeuronCores.

Data-parallel over batch: core b computes attention for batch element b.

Device algorithm (per core); matmuls in float32r (fp32 data, 11-bit-mantissa
RNE operand rounding in the PE, 3x the fp32 matmul rate), accumulation fp32:
  - Inputs pre-marshalled on host: aT = a.T  [C=1024, T=2048], Wqv = [Wq*scale | Wv]
    [1024, 128], Wk [1024, 64].
  - Projections: qT/vT from lhsT=Wqv tiles, kT from lhsT=Wk tiles, rhs = aT
    C-tiles; outputs land as qT [64, T] (partitions 0-63), vT (partitions
    64-127), kT [64, T].
  - v natural [T-tile, 65] built by PE transpose of vT 128-col chunks with an
    identity moving operand; column 64 is set to 1.0 (ones column).
  - Scores computed transposed: sT[tk, tq] via lhsT = kT tile [64, 128],
    rhs = qT chunk [64, 512] (contraction H=64).  exp on ScalarE directly from
    PSUM in [128, 1024] groups (2 k-tiles per op).  Causal mask = elementwise
    multiply with precomputed 0/1 masks on the 4 diagonal k-tiles per chunk.
  - PV: out_T/denom accumulate in one PSUM group: lhsT = [v | 1] [128, 65],
    rhs = expT group slices; row 64 of the [65, 512] accumulator is the
    softmax denominator (sum of exps).  No max-subtraction is needed: logits
    are ~N(0, ~1.5), max < ~10, exp is safely in fp32 range.
  - Normalize: reciprocal of denom row, broadcast across 64 partitions with a
    K=1 ones matmul, multiply, DMA out as outT [64, T].  Host transposes back.

T is processed in 4 chunks of 512 q-columns; aT is DMAed in T-quarters so
chunk j's entire dependency set (q, k, v cols <= 512(j+1)) arrives early and
compute overlaps the HBM stream.
"""

import sys

sys.path.insert(0, "/opt/trn_rl_repo")
sys.path.insert(0, "/root/.axon_site")

import numpy as np

import concourse.bass as bass
import concourse.mybir as mybir
import concourse.tile as tile
from concourse import bacc
from concourse import bass_utils

# If tracing is ever requested (e.g. BASS_TRACE=1), bass_utils imports
# antenv.axon_hooks, which this image lacks.  Register a ctypes-backed shim so
# that path degrades gracefully instead of raising ImportError.
try:
    from antenv import axon_hooks as _ah  # noqa: F401
except ImportError:
    try:
        import types as _types

        from trn_agent_boot.trn_boot import _ntff_profile_via_ctypes

        _mod = _types.ModuleType("antenv.axon_hooks")
        _hook = [None]
        _mod.set_axon_ntff_profile_hook = lambda h: _hook.__setitem__(0, h)
        _mod.get_axon_ntff_profile_hook = lambda: _hook[0]
        sys.modules["antenv.axon_hooks"] = _mod
        import antenv as _antenv

        _antenv.axon_hooks = _mod
        _mod.set_axon_ntff_profile_hook(
            _ntff_profile_via_ctypes("/opt/axon/libaxon_pjrt.so")
        )
    except Exception:
        pass

B, T, C, H = 8, 2048, 1024, 64
P = 128
NCT = C // P          # 8 C-tiles (contraction)
CHUNK = 512           # q-columns per chunk
NCH = T // CHUNK      # 4 chunks
NKT = T // P          # 16 k-tiles
SCALE = H ** -0.5
FP = mybir.dt.float32
FPR = mybir.dt.float32r   # 11-bit-mantissa RNE matmul mode, 3x faster than fp32

_cache = {}


def build_program():
    nc = bacc.Bacc("TRN2", target_bir_lowering=False, debug=False)

    aT = nc.dram_tensor("aT", [C, T], FPR, kind="ExternalInput").ap()
    wqv = nc.dram_tensor("wqv", [C, 2 * H], FPR, kind="ExternalInput").ap()
    wk = nc.dram_tensor("wk", [C, H], FPR, kind="ExternalInput").ap()
    idh = nc.dram_tensor("idh", [P, H], FPR, kind="ExternalInput").ap()
    m4 = nc.dram_tensor("m4", [P, 3 * P + CHUNK], FPR, kind="ExternalInput").ap()
    ones = nc.dram_tensor("ones", [P, H], FPR, kind="ExternalInput").ap()
    outT = nc.dram_tensor("outT", [H, T], FP, kind="ExternalOutput").ap()

    with tile.TileContext(nc) as tc:
        with (
            tc.tile_pool(name="const", bufs=1) as const_pool,
            tc.tile_pool(name="at", bufs=NCH) as at_pool,
            tc.tile_pool(name="qv", bufs=1) as qv_pool,
            tc.tile_pool(name="kt", bufs=1) as kt_pool,
            tc.tile_pool(name="v1", bufs=NKT) as v1_pool,
            tc.tile_pool(name="es", bufs=4) as e_pool,
            tc.tile_pool(name="norm", bufs=4) as norm_pool,
            tc.tile_pool(name="out", bufs=1) as out_pool,
            tc.tile_pool(name="ps_s", bufs=2, space="PSUM") as s_psum,
            tc.tile_pool(name="ps_proj", bufs=2, space="PSUM") as proj_psum,
            tc.tile_pool(name="ps_pv", bufs=1, space="PSUM") as pv_psum,
            tc.tile_pool(name="ps_small", bufs=1, space="PSUM") as small_psum,
        ):
            # ---- warm the ACT exp table + the PE clock during the DMA window
            warm = const_pool.tile([P, 8], FP, tag="warm")
            nc.scalar.activation(
                warm[:], warm[:], mybir.ActivationFunctionType.Exp
            )
            warm2 = const_pool.tile([P, CHUNK], FP, tag="warm2")
            nc.vector.memset(warm2[:], 0.0)
            warm_ps = small_psum.tile([P, CHUNK], FP, tag="small")
            for _ in range(6):
                nc.tensor.matmul(
                    warm_ps[:], warm2[:, :P], warm2[:], start=True, stop=True,
                )

            # ---- one strictly ordered sync DMA queue (arrival = need order) ----
            aT_r = aT.rearrange("(c p) t -> p c t", p=P)
            wqv_sb = const_pool.tile([P, NCT, 2 * H], FPR, tag="wqv")
            nc.sync.dma_start(wqv_sb[:], wqv.rearrange("(ko p) m -> p ko m", p=P))
            at_sb = {}

            def load_quarter(j, split=1):
                t_ = at_pool.tile([P, NCT, CHUNK], FPR, tag="at")
                step = NCT // split
                for h in range(split):
                    nc.sync.dma_start(
                        t_[:, h * step : (h + 1) * step, :],
                        aT_r[:, h * step : (h + 1) * step,
                             j * CHUNK : (j + 1) * CHUNK],
                    )
                at_sb[j] = t_

            load_quarter(0, split=2)
            wk_sb = const_pool.tile([P, NCT, H], FPR, tag="wk")
            nc.sync.dma_start(wk_sb[:], wk.rearrange("(ko p) m -> p ko m", p=P))
            idh_sb = const_pool.tile([P, H], FPR, tag="idh")
            nc.sync.dma_start(idh_sb[:], idh[:])
            ones_sb = const_pool.tile([P, H], FPR, tag="ones")
            nc.sync.dma_start(ones_sb[:], ones[:])
            m4_sb = const_pool.tile([P, 3 * P + CHUNK], FPR, tag="m4")
            nc.sync.dma_start(m4_sb[:], m4[:])
            for j in range(1, NCH):
                load_quarter(j)

            qv_sb = qv_pool.tile([P, T], FPR, tag="qv")   # q rows 0-63, vT rows 64-127
            kT_sb = kt_pool.tile([H, T], FPR, tag="kt")
            outT_sb = out_pool.tile([H, T], FP, tag="ot")
            v1 = {}

            for j in range(NCH):
                cs = slice(j * CHUNK, (j + 1) * CHUNK)

                # ---- projections for this chunk of T ----
                ps_qv = proj_psum.tile([P, CHUNK], FP, tag="proj")
                for c in range(NCT):
                    nc.tensor.matmul(
                        ps_qv[:], wqv_sb[:, c, :], at_sb[j][:, c, :],
                        start=(c == 0), stop=(c == NCT - 1),
                    )
                ps_k = proj_psum.tile([P, CHUNK], FP, tag="proj")
                for c in range(NCT):
                    nc.tensor.matmul(
                        ps_k[:H], wk_sb[:, c, :], at_sb[j][:, c, :],
                        start=(c == 0), stop=(c == NCT - 1),
                    )
                nc.vector.tensor_copy(qv_sb[:, cs], ps_qv[:])
                nc.vector.tensor_copy(kT_sb[:, cs], ps_k[:H])

                # ---- v natural tiles ([v | 1], PE transpose of vT chunks) ----
                for kt in range(4 * j, 4 * j + 4):
                    ps_t = small_psum.tile([P, H], FPR, tag="small")
                    nc.tensor.transpose(
                        ps_t[:],
                        qv_sb[H:P, kt * P : (kt + 1) * P],
                        idh_sb[H:P, :],
                    )
                    vt = v1_pool.tile([P, H + 1], FPR, tag="v1")
                    nc.vector.tensor_copy(vt[:, H : H + 1], ones_sb[:, :1])
                    nc.vector.tensor_copy(vt[:, :H], ps_t[:])
                    v1[kt] = vt

                # ---- attention ----
                ps_o = pv_psum.tile([H + 1, CHUNK], FP, tag="pv")
                nkt_j = 4 * j + 4          # k-tiles for this chunk (causal)
                # full (below-diagonal) k-tiles, pairs sharing one exp op
                for g in range(2 * j):
                    kts = [2 * g, 2 * g + 1]
                    ps_s = s_psum.tile([P, 2 * CHUNK], FP, tag="s")
                    for i, kt in enumerate(kts):
                        nc.tensor.matmul(
                            ps_s[:, i * CHUNK : (i + 1) * CHUNK],
                            kT_sb[:, kt * P : (kt + 1) * P],
                            qv_sb[:H, cs],
                            start=True, stop=True,
                        )
                    e_sb = e_pool.tile([P, 2 * CHUNK], FPR, tag="e")
                    nc.scalar.activation(
                        e_sb[:], ps_s[:], mybir.ActivationFunctionType.Exp
                    )
                    for i, kt in enumerate(kts):
                        nc.tensor.matmul(
                            ps_o[:],
                            v1[kt][:],
                            e_sb[:, i * CHUNK : (i + 1) * CHUNK],
                            start=(kt == 0), stop=(kt == nkt_j - 1),
                        )
                # diagonal k-tiles, narrowed to the causal region (cols >= off)
                for r in range(4):
                    kt = 4 * j + r
                    off = P * r if r < 3 else 2 * P   # keep matmul N >= 256
                    ncols = CHUNK - off
                    maskw = P * r - off + P
                    ps_s = s_psum.tile([P, 2 * CHUNK], FP, tag="s")
                    nc.tensor.matmul(
                        ps_s[:, :ncols],
                        kT_sb[:, kt * P : (kt + 1) * P],
                        qv_sb[:H, j * CHUNK + off : (j + 1) * CHUNK],
                        start=True, stop=True,
                    )
                    e_sb = e_pool.tile([P, 2 * CHUNK], FPR, tag="e")
                    nc.scalar.activation(
                        e_sb[:, :ncols], ps_s[:, :ncols],
                        mybir.ActivationFunctionType.Exp,
                    )
                    ms = 3 * P - (P * r - off)
                    nc.vector.tensor_mul(
                        e_sb[:, :maskw], e_sb[:, :maskw],
                        m4_sb[:, ms : ms + maskw],
                    )
                    nc.tensor.matmul(
                        ps_o[:, off:],
                        v1[kt][:],
                        e_sb[:, :ncols],
                        start=(kt == 0), stop=(kt == nkt_j - 1),
                    )

                # ---- normalize: out[h, tq] * 1/denom[tq] ----
                o_sb = norm_pool.tile([H + 1, CHUNK], FP, tag="osb")
                nc.vector.tensor_copy(o_sb[:], ps_o[:])
                rec_f = norm_pool.tile([H + 1, CHUNK], FP, tag="recf")
                nc.vector.reciprocal_approx_fast(rec_f[:], o_sb[:])
                rec = norm_pool.tile([H + 1, CHUNK], FPR, tag="rec")
                nc.vector.tensor_copy(rec[:], rec_f[:])
                ps_b = small_psum.tile([H, CHUNK], FP, tag="small")
                nc.tensor.matmul(
                    ps_b[:], ones_sb[H : H + 1, :], rec[H : H + 1, :],
                    start=True, stop=True,
                )
                nc.vector.tensor_mul(outT_sb[:, cs], o_sb[:H, :], ps_b[:])
                eng_out = nc.sync if j == NCH - 1 else nc.gpsimd
                eng_out.dma_start(outT[:, cs], outT_sb[:, cs])

    nc.compile()
    return nc


def _marshal(a, Wk, Wq, Wv):
    aT = np.ascontiguousarray(a.transpose(0, 2, 1))            # [B, C, T]
    wqv = np.ascontiguousarray(
        np.concatenate([Wq * np.float32(SCALE), Wv], axis=1)
    )                                                          # [C, 128]
    idh = np.zeros((P, H), np.float32)
    idh[H:P, :] = np.eye(H, dtype=np.float32)
    p = np.arange(P)[:, None]
    g = np.arange(3 * P + CHUNK)[None, :]
    m4 = (g >= p + 3 * P).astype(np.float32)
    ones = np.ones((P, H), np.float32)
    return aT, wqv, np.ascontiguousarray(Wk), idh, m4, ones


def kernel(a, Wk, Wq, Wv):
    a = np.asarray(a, np.float32)
    Wk = np.asarray(Wk, np.float32)
    Wq = np.asarray(Wq, np.float32)
    Wv = np.asarray(Wv, np.float32)
    if "nc" not in _cache:
        _cache["nc"] = build_program()
    nc = _cache["nc"]

    aT, wqv, wk, idh, m4, ones = _marshal(a, Wk, Wq, Wv)
    in_maps = [
        {"aT": aT[b], "wqv": wqv, "wk": wk, "idh": idh, "m4": m4, "ones": ones}
        for b in range(B)
    ]
    res = bass_utils.run_bass_kernel_spmd(nc, in_maps, core_ids=list(range(B)))
    out = np.stack(
        [np.ascontiguousarray(res.results[b]["outT"].T) for b in range(B)]
    )
    return out.astype(np.float32)


# revision 23
# speedup vs baseline: 1.0610x; 1.0610x over previous
"""Single-head causal attention (B=8, T=2048, C=1024, H=64) on 8 TRN2 NeuronCores.

Data-parallel over batch: core b computes attention for batch element b.

Device algorithm (per core); matmuls in float32r (fp32 data, 11-bit-mantissa
RNE operand rounding in the PE, 3x the fp32 matmul rate), accumulation fp32:
  - Inputs pre-marshalled on host: aT = a.T  [C=1024, T=2048], Wqv = [Wq*scale | Wv]
    [1024, 128], Wk [1024, 64].
  - Projections: qT/vT from lhsT=Wqv tiles, kT from lhsT=Wk tiles, rhs = aT
    C-tiles; outputs land as qT [64, T] (partitions 0-63), vT (partitions
    64-127), kT [64, T].
  - v natural [T-tile, 65] built by PE transpose of vT 128-col chunks with an
    identity moving operand; column 64 is set to 1.0 (ones column).
  - Scores computed transposed: sT[tk, tq] via lhsT = kT tile [64, 128],
    rhs = qT chunk [64, 512] (contraction H=64).  exp on ScalarE directly from
    PSUM ([128, 1024] per op for full k-tile pairs).  Diagonal k-tiles are
    narrowed to their causal region (matmul N kept >= 256 for full fp32r
    rate); the remaining triangular strip is zeroed by multiplying with a
    slice of one precomputed band-matrix mask.
  - PV: out_T/denom accumulate in one PSUM group: lhsT = [v | 1] [128, 65],
    rhs = expT group slices; row 64 of the [65, 512] accumulator is the
    softmax denominator (sum of exps).  No max-subtraction is needed: logits
    are ~N(0, ~1.5), max < ~10, exp is safely in fp32 range.
  - Normalize: reciprocal of denom row, broadcast across 64 partitions with a
    K=1 ones matmul, multiply, DMA out as outT [64, T].  Host transposes back.

T is processed in 4 chunks of 512 q-columns; aT is DMAed in T-quarters so
chunk j's entire dependency set (q, k, v cols <= 512(j+1)) arrives early and
compute overlaps the HBM stream.
"""

import sys

sys.path.insert(0, "/opt/trn_rl_repo")
sys.path.insert(0, "/root/.axon_site")

import numpy as np

import concourse.bass as bass
import concourse.mybir as mybir
import concourse.tile as tile
from concourse import bacc
from concourse import bass_utils

# If tracing is ever requested (e.g. BASS_TRACE=1), bass_utils imports
# antenv.axon_hooks, which this image lacks.  Register a ctypes-backed shim so
# that path degrades gracefully instead of raising ImportError.
try:
    from antenv import axon_hooks as _ah  # noqa: F401
except ImportError:
    try:
        import types as _types

        from trn_agent_boot.trn_boot import _ntff_profile_via_ctypes

        _mod = _types.ModuleType("antenv.axon_hooks")
        _hook = [None]
        _mod.set_axon_ntff_profile_hook = lambda h: _hook.__setitem__(0, h)
        _mod.get_axon_ntff_profile_hook = lambda: _hook[0]
        sys.modules["antenv.axon_hooks"] = _mod
        import antenv as _antenv

        _antenv.axon_hooks = _mod
        _mod.set_axon_ntff_profile_hook(
            _ntff_profile_via_ctypes("/opt/axon/libaxon_pjrt.so")
        )
    except Exception:
        pass

B, T, C, H = 8, 2048, 1024, 64
P = 128
NCT = C // P          # 8 C-tiles (contraction)
CHUNK = 512           # q-columns per chunk
NCH = T // CHUNK      # 4 chunks
NKT = T // P          # 16 k-tiles
SCALE = H ** -0.5
FP = mybir.dt.float32
FPR = mybir.dt.float32r   # 11-bit-mantissa RNE matmul mode, 3x faster than fp32

_cache = {}


def build_program():
    nc = bacc.Bacc("TRN2", target_bir_lowering=False, debug=False)

    aT = nc.dram_tensor("aT", [C, T], FPR, kind="ExternalInput").ap()
    wqv = nc.dram_tensor("wqv", [C, 2 * H], FPR, kind="ExternalInput").ap()
    wk = nc.dram_tensor("wk", [C, H], FPR, kind="ExternalInput").ap()
    idh = nc.dram_tensor("idh", [P, H], FPR, kind="ExternalInput").ap()
    m4 = nc.dram_tensor("m4", [P, 3 * P + CHUNK], FPR, kind="ExternalInput").ap()
    ones = nc.dram_tensor("ones", [P, H], FPR, kind="ExternalInput").ap()
    outT = nc.dram_tensor("outT", [H, T], FP, kind="ExternalOutput").ap()

    with tile.TileContext(nc) as tc:
        with (
            tc.tile_pool(name="const", bufs=1) as const_pool,
            tc.tile_pool(name="at", bufs=NCH) as at_pool,
            tc.tile_pool(name="qv", bufs=1) as qv_pool,
            tc.tile_pool(name="kt", bufs=1) as kt_pool,
            tc.tile_pool(name="v1", bufs=NKT) as v1_pool,
            tc.tile_pool(name="es", bufs=3) as e_pool,
            tc.tile_pool(name="norm", bufs=4) as norm_pool,
            tc.tile_pool(name="out", bufs=1) as out_pool,
            tc.tile_pool(name="ps_s", bufs=2, space="PSUM") as s_psum,
            tc.tile_pool(name="ps_proj", bufs=2, space="PSUM") as proj_psum,
            tc.tile_pool(name="ps_pv", bufs=1, space="PSUM") as pv_psum,
            tc.tile_pool(name="ps_small", bufs=1, space="PSUM") as small_psum,
        ):
            # ---- warm the ACT exp table + the PE clock during the DMA window
            warm = const_pool.tile([P, 8], FP, tag="warm")
            nc.scalar.activation(
                warm[:], warm[:], mybir.ActivationFunctionType.Exp
            )
            warm2 = const_pool.tile([P, CHUNK], FP, tag="warm2")
            nc.vector.memset(warm2[:], 0.0)
            warm_ps = small_psum.tile([P, CHUNK], FP, tag="small")
            for _ in range(6):
                nc.tensor.matmul(
                    warm_ps[:], warm2[:, :P], warm2[:], start=True, stop=True,
                )

            # ---- one strictly ordered sync DMA queue (arrival = need order) ----
            aT_r = aT.rearrange("(c p) t -> p c t", p=P)
            wqv_sb = const_pool.tile([P, NCT, 2 * H], FPR, tag="wqv")
            nc.sync.dma_start(wqv_sb[:], wqv.rearrange("(ko p) m -> p ko m", p=P))
            at_sb = {}

            def load_quarter(j, split=1):
                t_ = at_pool.tile([P, NCT, CHUNK], FPR, tag="at")
                step = NCT // split
                for h in range(split):
                    nc.sync.dma_start(
                        t_[:, h * step : (h + 1) * step, :],
                        aT_r[:, h * step : (h + 1) * step,
                             j * CHUNK : (j + 1) * CHUNK],
                    )
                at_sb[j] = t_

            load_quarter(0, split=2)
            wk_sb = const_pool.tile([P, NCT, H], FPR, tag="wk")
            nc.sync.dma_start(wk_sb[:], wk.rearrange("(ko p) m -> p ko m", p=P))
            idh_sb = const_pool.tile([P, H], FPR, tag="idh")
            nc.sync.dma_start(idh_sb[:], idh[:])
            ones_sb = const_pool.tile([P, H], FPR, tag="ones")
            nc.sync.dma_start(ones_sb[:], ones[:])
            m4_sb = const_pool.tile([P, 3 * P + CHUNK], FPR, tag="m4")
            nc.sync.dma_start(m4_sb[:], m4[:])
            for j in range(1, NCH):
                load_quarter(j)

            qv_sb = qv_pool.tile([P, T], FPR, tag="qv")   # q rows 0-63, vT rows 64-127
            kT_sb = kt_pool.tile([H, T], FPR, tag="kt")
            outT_sb = out_pool.tile([H, T], FP, tag="ot")
            v1 = {}

            for j in range(NCH):
                cs = slice(j * CHUNK, (j + 1) * CHUNK)

                # ---- projections for this chunk of T ----
                ps_qv = proj_psum.tile([P, CHUNK], FP, tag="proj")
                for c in range(NCT):
                    nc.tensor.matmul(
                        ps_qv[:], wqv_sb[:, c, :], at_sb[j][:, c, :],
                        start=(c == 0), stop=(c == NCT - 1),
                    )
                ps_k = proj_psum.tile([P, CHUNK], FP, tag="proj")
                for c in range(NCT):
                    nc.tensor.matmul(
                        ps_k[:H], wk_sb[:, c, :], at_sb[j][:, c, :],
                        start=(c == 0), stop=(c == NCT - 1),
                    )
                nc.vector.tensor_copy(qv_sb[:, cs], ps_qv[:])
                nc.vector.tensor_copy(kT_sb[:, cs], ps_k[:H])

                # ---- v natural tiles ([v | 1], PE transpose of vT chunks) ----
                for kt in range(4 * j, 4 * j + 4):
                    ps_t = small_psum.tile([P, H], FPR, tag="small")
                    nc.tensor.transpose(
                        ps_t[:],
                        qv_sb[H:P, kt * P : (kt + 1) * P],
                        idh_sb[H:P, :],
                    )
                    vt = v1_pool.tile([P, H + 1], FPR, tag="v1")
                    nc.vector.tensor_copy(vt[:, H : H + 1], ones_sb[:, :1])
                    nc.vector.tensor_copy(vt[:, :H], ps_t[:])
                    v1[kt] = vt

                # ---- attention ----
                ps_o = pv_psum.tile([H + 1, CHUNK], FP, tag="pv")
                nkt_j = 4 * j + 4          # k-tiles for this chunk (causal)
                # full (below-diagonal) k-tiles, pairs sharing one exp op
                for g in range(2 * j):
                    kts = [2 * g, 2 * g + 1]
                    ps_s = s_psum.tile([P, 2 * CHUNK], FP, tag="s")
                    for i, kt in enumerate(kts):
                        nc.tensor.matmul(
                            ps_s[:, i * CHUNK : (i + 1) * CHUNK],
                            kT_sb[:, kt * P : (kt + 1) * P],
                            qv_sb[:H, cs],
                            start=True, stop=True,
                        )
                    e_sb = e_pool.tile([P, 2 * CHUNK], FPR, tag="e")
                    nc.scalar.activation(
                        e_sb[:], ps_s[:], mybir.ActivationFunctionType.Exp
                    )
                    for i, kt in enumerate(kts):
                        nc.tensor.matmul(
                            ps_o[:],
                            v1[kt][:],
                            e_sb[:, i * CHUNK : (i + 1) * CHUNK],
                            start=(kt == 0), stop=(kt == nkt_j - 1),
                        )
                # diagonal k-tiles, narrowed to the causal region (cols >= off)
                for r in range(4):
                    kt = 4 * j + r
                    off = P * r if r < 3 else 2 * P   # keep matmul N >= 256
                    ncols = CHUNK - off
                    maskw = P * r - off + P
                    ps_s = s_psum.tile([P, 2 * CHUNK], FP, tag="s")
                    nc.tensor.matmul(
                        ps_s[:, :ncols],
                        kT_sb[:, kt * P : (kt + 1) * P],
                        qv_sb[:H, j * CHUNK + off : (j + 1) * CHUNK],
                        start=True, stop=True,
                    )
                    e_sb = e_pool.tile([P, 2 * CHUNK], FPR, tag="e")
                    nc.scalar.activation(
                        e_sb[:, :ncols], ps_s[:, :ncols],
                        mybir.ActivationFunctionType.Exp,
                    )
                    ms = 3 * P - (P * r - off)
                    nc.vector.tensor_mul(
                        e_sb[:, :maskw], e_sb[:, :maskw],
                        m4_sb[:, ms : ms + maskw],
                    )
                    nc.tensor.matmul(
                        ps_o[:, off:],
                        v1[kt][:],
                        e_sb[:, :ncols],
                        start=(kt == 0), stop=(kt == nkt_j - 1),
                    )

                # ---- normalize: out[h, tq] * 1/denom[tq] ----
                o_sb = norm_pool.tile([H + 1, CHUNK], FP, tag="osb")
                nc.vector.tensor_copy(o_sb[:], ps_o[:])
                rec_f = norm_pool.tile([H + 1, CHUNK], FP, tag="recf")
                nc.vector.reciprocal_approx_fast(rec_f[:], o_sb[:])
                rec = norm_pool.tile([H + 1, CHUNK], FPR, tag="rec")
                nc.vector.tensor_copy(rec[:], rec_f[:])
                ps_b = small_psum.tile([H, CHUNK], FP, tag="small")
                nc.tensor.matmul(
                    ps_b[:], ones_sb[H : H + 1, :], rec[H : H + 1, :],
                    start=True, stop=True,
                )
                nc.vector.tensor_mul(outT_sb[:, cs], o_sb[:H, :], ps_b[:])
                eng_out = nc.sync if j == NCH - 1 else nc.gpsimd
                eng_out.dma_start(outT[:, cs], outT_sb[:, cs])

    nc.compile()
    return nc


def _marshal(a, Wk, Wq, Wv):
    aT = np.ascontiguousarray(a.transpose(0, 2, 1))            # [B, C, T]
    wqv = np.ascontiguousarray(
        np.concatenate([Wq * np.float32(SCALE), Wv], axis=1)
    )                                                          # [C, 128]
    idh = np.zeros((P, H), np.float32)
    idh[H:P, :] = np.eye(H, dtype=np.float32)
    p = np.arange(P)[:, None]
    g = np.arange(3 * P + CHUNK)[None, :]
    m4 = (g >= p + 3 * P).astype(np.float32)
    ones = np.ones((P, H), np.float32)
    return aT, wqv, np.ascontiguousarray(Wk), idh, m4, ones


def kernel(a, Wk, Wq, Wv):
    a = np.asarray(a, np.float32)
    Wk = np.asarray(Wk, np.float32)
    Wq = np.asarray(Wq, np.float32)
    Wv = np.asarray(Wv, np.float32)
    if "nc" not in _cache:
        _cache["nc"] = build_program()
    nc = _cache["nc"]

    aT, wqv, wk, idh, m4, ones = _marshal(a, Wk, Wq, Wv)
    in_maps = [
        {"aT": aT[b], "wqv": wqv, "wk": wk, "idh": idh, "m4": m4, "ones": ones}
        for b in range(B)
    ]
    res = bass_utils.run_bass_kernel_spmd(nc, in_maps, core_ids=list(range(B)))
    out = np.stack(
        [np.ascontiguousarray(res.results[b]["outT"].T) for b in range(B)]
    )
    return out.astype(np.float32)
